# revision 4
# baseline (speedup 1.0000x reference)
"""Cumulative-probability head on 8 Trainium2 NeuronCores.

out[b, j] = sum_{i<=j} relu(x @ W_h^T + b_h)[b, i] + (x @ W_base^T + b_base)[b]

Data-parallel: x sharded along batch (1024 rows/core), weights replicated.

Per-core strategy (fp8 DoubleRow):
  - x and W are quantized host-side to TRN fp8-e4m3 (ml_dtypes.float8_e4m3,
    matching TRN FP8_EXP4: max normal 240) with power-of-2 scales
    Sx=16, Sw=512. The matmul runs in MatmulPerfMode.DoubleRow (2 fp8
    MACs/cell/cycle -> 157 TF/s), accumulating S*x@W in fp32 PSUM.
  - Contraction 2048 = 8 chunks x (128 partitions x 2 doublerow slots):
    k = 256*c + 2*p + i. Tiles are [128, 2, N]; lhsT = x chunk (stationary,
    batch on free dim), rhs = W chunk (moving, T on free dim).
  - Hazard matmul N=512 fills exactly one PSUM bank; the base column rides
    as a tiny N=2 matmul into a shared [128,16] PSUM tile (one bank, one
    2-col accumulation region per 128-row batch tile).
  - Batch processed in 2 waves of 512 rows (4 b-tiles each), chunk-outer
    loop so early chunks feed the PE while later chunks stream in.
    PSUM: 6 hazard banks (ring) + 1 base bank.
  - Post per b-tile: DVE adds S*bias into PSUM in place, ScalarE applies
    Relu with scale 1/S into bf16, base col gets Identity(scale)+b_base,
    DVE tensor_tensor_scan (fp32 internal state) does the inclusive
    cumsum with the base as initial state, bf16 output DMA'd out.
  - Input DMAs spread over Sync/Scalar HWDGE + GPSIMD SWDGE rings,
    k-ordered; wave-1 x streams during wave-0 compute.
"""

import numpy as np
import ml_dtypes

import concourse.bass as bass
import concourse.tile as tile
from concourse import bacc, mybir
from concourse.bass_utils import run_bass_kernel_spmd

B, D, T = 8192, 2048, 512
NCORES = 8
BLOC = B // NCORES            # 1024 rows per core
WB = BLOC // 2                # 512 rows per wave
NBW = WB // 128               # 4 b-tiles per wave
NCH = D // 256                # 8 contraction chunks (256 = 128 x 2 doublerow)
TP = 516                      # padded W width: 512 hazard + base + 3 zero
SX = 16.0                     # x fp8 scale
SW = 512.0                    # W fp8 scale
S = SX * SW

F32 = mybir.dt.float32
BF16 = mybir.dt.bfloat16
F8 = mybir.dt.float8e4

F8NP = ml_dtypes.float8_e4m3
BF16NP = ml_dtypes.bfloat16


def _build_program():
    nc = bacc.Bacc("TRN2", target_bir_lowering=False, debug=False)

    xt_d = nc.dram_tensor("xt", [2, D, WB], F8, kind="ExternalInput")
    wt_d = nc.dram_tensor("wt", [D, TP], F8, kind="ExternalInput")
    bias_d = nc.dram_tensor("bias", [1, TP], BF16, kind="ExternalInput")
    out_d = nc.dram_tensor("out", [BLOC, T], BF16, kind="ExternalOutput")

    DR = mybir.MatmulPerfMode.DoubleRow
    Relu = mybir.ActivationFunctionType.Relu
    Ident = mybir.ActivationFunctionType.Identity

    with tile.TileContext(nc) as tc:
        with (
            tc.tile_pool(name="consts", bufs=1) as consts,
            tc.tile_pool(name="wt", bufs=1) as wtp,
            tc.tile_pool(name="xt", bufs=1) as xtp,
            tc.tile_pool(name="haz", bufs=4) as hazp,
            tc.tile_pool(name="outp", bufs=4) as outp,
            tc.tile_pool(name="ps", bufs=6, space="PSUM") as psp,
            tc.tile_pool(name="psb", bufs=1, space="PSUM") as psbp,
        ):
            zeros = consts.tile([128, T], BF16, tag="zeros")
            nc.vector.memset(zeros, 0.0)
            bias_bc = consts.tile([128, TP], BF16, tag="bias")

            rings = [nc.sync, nc.scalar, nc.gpsimd]
            # Ring choice per transfer: HWDGE rings (sync=0, scalar=1)
            # carry the chunks that gate the PE ramp; SWDGE (2) takes
            # late-need traffic.
            WT_RING = [0, 2, 1, 0, 2, 1, 0, 2]
            X0_RING = [1, 0, 2, 1, 0, 2, 1, 0]
            X1_RING = [1, 0, 2, 1, 0, 2, 1, 0]

            wt_tiles = []
            xt_tiles = [[None] * NCH for _ in range(2)]
            for c in range(NCH):
                w = wtp.tile([128, 2, TP], F8, tag=f"wt{c}")
                rings[WT_RING[c]].dma_start(
                    out=w, in_=wt_d[256 * c : 256 * (c + 1), :]
                )
                wt_tiles.append(w)
                xk = xtp.tile([128, 2, WB], F8, tag=f"x0_{c}")
                rings[X0_RING[c]].dma_start(
                    out=xk, in_=xt_d[0, 256 * c : 256 * (c + 1), :]
                )
                xt_tiles[0][c] = xk
            # Bias row broadcast to 128 partitions via stride-0 partition
            # DMA read (engines can't read stride-0 partition APs; DMA can).
            bsrc = bias_d[0:1, :]
            nc.gpsimd.dma_start(
                out=bias_bc,
                in_=bass.AP(
                    tensor=bsrc.tensor,
                    offset=bsrc.offset,
                    ap=[[0, 128]] + list(bsrc.ap[1:]),
                ),
            )
            for c in range(NCH):
                xk = xtp.tile([128, 2, WB], F8, tag=f"x1_{c}")
                rings[X1_RING[c]].dma_start(
                    out=xk, in_=xt_d[1, 256 * c : 256 * (c + 1), :]
                )
                xt_tiles[1][c] = xk

            base_ps = psbp.tile([128, 2 * NCORES], F32, tag="bps")
            out_rings = [nc.scalar, nc.sync]

            def mm_pair(ps, wv, c, bl, b):
                lhsT = xt_tiles[wv][c][:, :, 128 * bl : 128 * (bl + 1)]
                nc.tensor.matmul(
                    ps[:],
                    lhsT,
                    wt_tiles[c][:, :, 0:T],
                    start=(c == 0),
                    stop=(c == NCH - 1),
                    perf_mode=DR,
                )
                nc.tensor.matmul(
                    base_ps[:, 2 * b : 2 * b + 2],
                    lhsT,
                    wt_tiles[c][:, :, T : T + 2],
                    start=(c == 0),
                    stop=(c == NCH - 1),
                    perf_mode=DR,
                )

            def drain(ps, b):
                # psum += S*bias (in place), relu with descale into bf16,
                # then inclusive scan (split in two chained halves so the
                # first half's output DMA overlaps the second half's scan).
                nc.vector.tensor_add(ps[:], ps[:], bias_bc[:, 0:T])
                haz = hazp.tile([128, T], BF16, tag="haz", name=f"haz{b}")
                nc.scalar.activation(out=haz, in_=ps[:], func=Relu, scale=1.0 / S)
                baset = hazp.tile([128, 1], BF16, tag="base", name=f"base{b}")
                nc.scalar.activation(
                    out=baset,
                    in_=base_ps[:, 2 * b : 2 * b + 1],
                    func=Ident,
                    scale=1.0 / S,
                    bias=bias_bc[:, T : T + 1],
                )
                cum = outp.tile([128, T], BF16, tag="cum", name=f"cum{b}")
                H = T // 2
                nc.vector.tensor_tensor_scan(
                    out=cum[:, 0:H],
                    data0=haz[:, 0:H],
                    data1=zeros[:, 0:H],
                    initial=baset,
                    op0=mybir.AluOpType.add,
                    op1=mybir.AluOpType.add,
                )
                out_rings[b % 2].dma_start(
                    out=out_d[128 * b : 128 * (b + 1), 0:H], in_=cum[:, 0:H]
                )
                nc.vector.tensor_tensor_scan(
                    out=cum[:, H:T],
                    data0=haz[:, H:T],
                    data1=zeros[:, H:T],
                    initial=cum[:, H - 1 : H],
                    op0=mybir.AluOpType.add,
                    op1=mybir.AluOpType.add,
                )
                out_rings[b % 2].dma_start(
                    out=out_d[128 * b : 128 * (b + 1), H:T], in_=cum[:, H:T]
                )

            # Wave 0 (b-tiles 0..3): chunk-outer so the PE paces with the
            # incoming x/W chunk stream; all four accumulations finish
            # together and drain while wave 1 computes.
            ps0 = [
                psp.tile([128, T], F32, tag="ps", name=f"ps_0_{i}")
                for i in range(NBW)
            ]
            for c in range(NCH):
                for bl in range(NBW):
                    mm_pair(ps0[bl], 0, c, bl, bl)
            for bl in range(NBW):
                drain(ps0[bl], bl)

            # Wave 1 (b-tiles 4..7): all inputs are resident by now, so go
            # b-outer — each tile's accumulation stops early and its drain
            # overlaps the next tile's matmuls, instead of four full drain
            # chains serializing after the last matmul.
            for bl in range(NBW):
                b = NBW + bl
                ps = psp.tile([128, T], F32, tag="ps", name=f"ps_1_{bl}")
                for c in range(NCH):
                    mm_pair(ps, 1, c, bl, b)
                drain(ps, b)

    nc.compile()
    return nc


_NC_CACHE = None


def prep_in_maps(x, W_hazard, b_hazard, W_base, b_base):
    x = np.asarray(x, np.float32)
    Wh = np.asarray(W_hazard, np.float32)
    bh = np.asarray(b_hazard, np.float32)
    Wb = np.asarray(W_base, np.float32).reshape(1, D)
    bb = np.asarray(b_base, np.float32).reshape(1)

    wt = np.zeros((D, TP), np.float32)
    wt[:, 0 : T + 1] = np.concatenate([Wh, Wb], axis=0).T * SW
    np.clip(wt, -240.0, 240.0, out=wt)
    wt8 = wt.astype(F8NP)

    bias = np.zeros((1, TP), np.float32)
    bias[0, 0:T] = bh * S
    bias[0, T] = bb[0]
    bias16 = bias.astype(BF16NP)

    x8 = np.clip(x * SX, -240.0, 240.0).astype(F8NP)  # [B, D]
    in_maps = []
    for i in range(NCORES):
        xs = x8[BLOC * i : BLOC * (i + 1)]  # [1024, D]
        xt = np.ascontiguousarray(xs.T.reshape(D, 2, WB).transpose(1, 0, 2))
        in_maps.append({"xt": xt, "wt": wt8, "bias": bias16})
    return in_maps


def kernel(x, W_hazard, b_hazard, W_base, b_base):
    global _NC_CACHE
    if _NC_CACHE is None:
        _NC_CACHE = _build_program()
    in_maps = prep_in_maps(x, W_hazard, b_hazard, W_base, b_base)
    res = run_bass_kernel_spmd(_NC_CACHE, in_maps, list(range(NCORES)))
    return np.concatenate(
        [res.results[i]["out"].astype(np.float32) for i in range(NCORES)], axis=0
    )


# revision 9
# speedup vs baseline: 1.0194x; 1.0194x over previous
"""Cumulative-probability head on 8 Trainium2 NeuronCores.

out[b, j] = sum_{i<=j} relu(x @ W_h^T + b_h)[b, i] + (x @ W_base^T + b_base)[b]

Data-parallel: x sharded along batch (1024 rows/core), weights replicated.

Per-core strategy (fp8 DoubleRow):
  - x and W are quantized host-side to TRN fp8-e4m3 (ml_dtypes.float8_e4m3,
    matching TRN FP8_EXP4: max normal 240) with power-of-2 scales
    Sx=16, Sw=512. The matmul runs in MatmulPerfMode.DoubleRow (2 fp8
    MACs/cell/cycle -> 157 TF/s), accumulating S*x@W in fp32 PSUM.
  - Contraction 2048 = 8 chunks x (128 partitions x 2 doublerow slots):
    k = 256*c + 2*p + i. Tiles are [128, 2, N]; lhsT = x chunk (stationary,
    batch on free dim), rhs = W chunk (moving, T on free dim).
  - Hazard matmul N=512 fills exactly one PSUM bank; the base column rides
    as a tiny N=2 matmul into a shared [128,16] PSUM tile (one bank, one
    2-col accumulation region per 128-row batch tile).
  - Batch processed in 2 waves of 512 rows (4 b-tiles each), chunk-outer
    loop so early chunks feed the PE while later chunks stream in.
    PSUM: 6 hazard banks (ring) + 1 base bank.
  - Post per b-tile: DVE adds S*bias into PSUM in place, ScalarE applies
    Relu with scale 1/S into bf16, base col gets Identity(scale)+b_base,
    DVE tensor_tensor_scan (fp32 internal state) does the inclusive
    cumsum with the base as initial state, bf16 output DMA'd out.
  - Input DMAs spread over Sync/Scalar HWDGE + GPSIMD SWDGE rings,
    k-ordered; wave-1 x streams during wave-0 compute.
"""

import numpy as np
import ml_dtypes

import concourse.bass as bass
import concourse.tile as tile
from concourse import bacc, mybir
from concourse.bass_utils import run_bass_kernel_spmd

B, D, T = 8192, 2048, 512
NCORES = 8
BLOC = B // NCORES            # 1024 rows per core
WB = BLOC // 2                # 512 rows per wave
NBW = WB // 128               # 4 b-tiles per wave
NCH = D // 256                # 8 contraction chunks (256 = 128 x 2 doublerow)
TP = 516                      # padded W width: 512 hazard + base + 3 zero
SX = 16.0                     # x fp8 scale
SW = 512.0                    # W fp8 scale
S = SX * SW

F32 = mybir.dt.float32
BF16 = mybir.dt.bfloat16
F8 = mybir.dt.float8e4

F8NP = ml_dtypes.float8_e4m3
BF16NP = ml_dtypes.bfloat16


def _build_program():
    nc = bacc.Bacc("TRN2", target_bir_lowering=False, debug=False)

    xt_d = nc.dram_tensor("xt", [2, D, WB], F8, kind="ExternalInput")
    wt_d = nc.dram_tensor("wt", [D, TP], F8, kind="ExternalInput")
    bias_d = nc.dram_tensor("bias", [1, TP], BF16, kind="ExternalInput")
    out_d = nc.dram_tensor("out", [BLOC, T], BF16, kind="ExternalOutput")

    DR = mybir.MatmulPerfMode.DoubleRow
    Relu = mybir.ActivationFunctionType.Relu
    Ident = mybir.ActivationFunctionType.Identity

    with tile.TileContext(nc) as tc:
        with (
            tc.tile_pool(name="consts", bufs=1) as consts,
            tc.tile_pool(name="wt", bufs=1) as wtp,
            tc.tile_pool(name="xt", bufs=1) as xtp,
            tc.tile_pool(name="haz", bufs=4) as hazp,
            tc.tile_pool(name="outp", bufs=4) as outp,
            tc.tile_pool(name="ps", bufs=5, space="PSUM") as psp,
            tc.tile_pool(name="psb", bufs=1, space="PSUM") as psbp,
            tc.tile_pool(name="psb1", bufs=2, space="PSUM") as psb1p,
        ):
            zeros = consts.tile([128, T], BF16, tag="zeros")
            nc.vector.memset(zeros, 0.0)
            bias_bc = consts.tile([128, TP], BF16, tag="bias")

            rings = [nc.sync, nc.scalar, nc.gpsimd]
            # Ring choice per transfer: HWDGE rings (sync=0, scalar=1)
            # carry the chunks that gate the PE ramp; SWDGE (2) takes
            # late-need traffic.
            WT_RING = [0, 2, 1, 0, 2, 1, 0, 2]
            X0_RING = [1, 0, 2, 1, 0, 2, 1, 0]
            X1_RING = [1, 0, 2, 1, 0, 2, 1, 0]

            wt_tiles = []
            xt_tiles = [[None] * NCH for _ in range(2)]
            for c in range(NCH):
                w = wtp.tile([128, 2, TP], F8, tag=f"wt{c}")
                if c == 0:
                    # Chunk 0 gates the PE start: split its transfers in
                    # halves across the rings so it lands sooner.
                    nc.sync.dma_start(
                        out=w[:, :, 0:258], in_=wt_d[0:256, 0:258]
                    )
                    nc.scalar.dma_start(
                        out=w[:, :, 258:TP], in_=wt_d[0:256, 258:TP]
                    )
                else:
                    rings[WT_RING[c]].dma_start(
                        out=w, in_=wt_d[256 * c : 256 * (c + 1), :]
                    )
                wt_tiles.append(w)
                xk = xtp.tile([128, 2, WB], F8, tag=f"x0_{c}")
                if c == 0:
                    nc.gpsimd.dma_start(
                        out=xk[:, :, 0:256], in_=xt_d[0, 0:256, 0:256]
                    )
                    nc.sync.dma_start(
                        out=xk[:, :, 256:WB], in_=xt_d[0, 0:256, 256:WB]
                    )
                else:
                    rings[X0_RING[c]].dma_start(
                        out=xk, in_=xt_d[0, 256 * c : 256 * (c + 1), :]
                    )
                xt_tiles[0][c] = xk
            # Bias row broadcast to 128 partitions via stride-0 partition
            # DMA read (engines can't read stride-0 partition APs; DMA can).
            bsrc = bias_d[0:1, :]
            nc.gpsimd.dma_start(
                out=bias_bc,
                in_=bass.AP(
                    tensor=bsrc.tensor,
                    offset=bsrc.offset,
                    ap=[[0, 128]] + list(bsrc.ap[1:]),
                ),
            )
            for c in range(NCH):
                xk = xtp.tile([128, 2, WB], F8, tag=f"x1_{c}")
                rings[X1_RING[c]].dma_start(
                    out=xk, in_=xt_d[1, 256 * c : 256 * (c + 1), :]
                )
                xt_tiles[1][c] = xk

            base_ps0 = psbp.tile([128, 2 * NBW], F32, tag="bps")
            out_rings = [nc.scalar, nc.sync]

            def mm_pair(ps, bps, wv, c, bl):
                lhsT = xt_tiles[wv][c][:, :, 128 * bl : 128 * (bl + 1)]
                nc.tensor.matmul(
                    ps[:],
                    lhsT,
                    wt_tiles[c][:, :, 0:T],
                    start=(c == 0),
                    stop=(c == NCH - 1),
                    perf_mode=DR,
                )
                nc.tensor.matmul(
                    bps,
                    lhsT,
                    wt_tiles[c][:, :, T : T + 2],
                    start=(c == 0),
                    stop=(c == NCH - 1),
                    perf_mode=DR,
                )

            def drain(ps, bps, b):
                # psum += S*bias (in place), relu with descale into bf16,
                # then inclusive scan (split in two chained halves so the
                # first half's output DMA overlaps the second half's scan).
                nc.vector.tensor_add(ps[:], ps[:], bias_bc[:, 0:T])
                haz = hazp.tile([128, T], BF16, tag="haz", name=f"haz{b}")
                nc.scalar.activation(out=haz, in_=ps[:], func=Relu, scale=1.0 / S)
                baset = hazp.tile([128, 1], BF16, tag="base", name=f"base{b}")
                nc.scalar.activation(
                    out=baset,
                    in_=bps[:, 0:1],
                    func=Ident,
                    scale=1.0 / S,
                    bias=bias_bc[:, T : T + 1],
                )
                cum = outp.tile([128, T], BF16, tag="cum", name=f"cum{b}")
                H = T // 2
                nc.vector.tensor_tensor_scan(
                    out=cum[:, 0:H],
                    data0=haz[:, 0:H],
                    data1=zeros[:, 0:H],
                    initial=baset,
                    op0=mybir.AluOpType.add,
                    op1=mybir.AluOpType.add,
                )
                out_rings[b % 2].dma_start(
                    out=out_d[128 * b : 128 * (b + 1), 0:H], in_=cum[:, 0:H]
                )
                nc.vector.tensor_tensor_scan(
                    out=cum[:, H:T],
                    data0=haz[:, H:T],
                    data1=zeros[:, H:T],
                    initial=cum[:, H - 1 : H],
                    op0=mybir.AluOpType.add,
                    op1=mybir.AluOpType.add,
                )
                out_rings[b % 2].dma_start(
                    out=out_d[128 * b : 128 * (b + 1), H:T], in_=cum[:, H:T]
                )

            # Wave 0 (b-tiles 0..3): chunk-outer so the PE paces with the
            # incoming x/W chunk stream; all four accumulations finish
            # together and drain while wave 1 computes.
            ps0 = [
                psp.tile([128, T], F32, tag="ps", name=f"ps_0_{i}")
                for i in range(NBW)
            ]
            for c in range(NCH):
                for bl in range(NBW):
                    mm_pair(ps0[bl], base_ps0[:, 2 * bl : 2 * bl + 2], 0, c, bl)
            for bl in range(NBW):
                drain(ps0[bl], base_ps0[:, 2 * bl : 2 * bl + 2], bl)

            # Wave 1 (b-tiles 4..7): all inputs are resident by now, so go
            # b-outer — each tile's accumulation stops early and its drain
            # overlaps the next tile's matmuls, instead of four full drain
            # chains serializing after the last matmul. Each b-tile gets its
            # own base PSUM tile so a drain's read doesn't block the next
            # tile's base matmul (tile-granularity dependency tracking).
            for bl in range(NBW):
                b = NBW + bl
                ps = psp.tile([128, T], F32, tag="ps", name=f"ps_1_{bl}")
                bps = psb1p.tile([128, 2], F32, tag="bps1", name=f"bps_1_{bl}")
                for c in range(NCH):
                    mm_pair(ps, bps, 1, c, bl)
                drain(ps, bps, b)

    nc.compile()
    return nc


_NC_CACHE = None


def prep_in_maps(x, W_hazard, b_hazard, W_base, b_base):
    x = np.asarray(x, np.float32)
    Wh = np.asarray(W_hazard, np.float32)
    bh = np.asarray(b_hazard, np.float32)
    Wb = np.asarray(W_base, np.float32).reshape(1, D)
    bb = np.asarray(b_base, np.float32).reshape(1)

    wt = np.zeros((D, TP), np.float32)
    wt[:, 0 : T + 1] = np.concatenate([Wh, Wb], axis=0).T * SW
    np.clip(wt, -240.0, 240.0, out=wt)
    wt8 = wt.astype(F8NP)

    bias = np.zeros((1, TP), np.float32)
    bias[0, 0:T] = bh * S
    bias[0, T] = bb[0]
    bias16 = bias.astype(BF16NP)

    x8 = np.clip(x * SX, -240.0, 240.0).astype(F8NP)  # [B, D]
    in_maps = []
    for i in range(NCORES):
        xs = x8[BLOC * i : BLOC * (i + 1)]  # [1024, D]
        xt = np.ascontiguousarray(xs.T.reshape(D, 2, WB).transpose(1, 0, 2))
        in_maps.append({"xt": xt, "wt": wt8, "bias": bias16})
    return in_maps


def kernel(x, W_hazard, b_hazard, W_base, b_base):
    global _NC_CACHE
    if _NC_CACHE is None:
        _NC_CACHE = _build_program()
    in_maps = prep_in_maps(x, W_hazard, b_hazard, W_base, b_base)
    res = run_bass_kernel_spmd(_NC_CACHE, in_maps, list(range(NCORES)))
    return np.concatenate(
        [res.results[i]["out"].astype(np.float32) for i in range(NCORES)], axis=0
    )
